# revision 1
# baseline (speedup 1.0000x reference)
"""GCN layer on 8 TRN2 NeuronCores (Bass/Tile).

out = segment_sum(edge_vals[:,None] * (X @ W)[edge_col], edge_row, N)

Strategy (1D destination-node sharding):
  - Host: cast/transpose X -> XT bf16 (replicated to all 8 cores). Partition
    edges by destination shard (6250 rows/core), group by destination window
    (128 rows), and within each window split by source-half (lo/hi) so the
    int16 dma_gather indices stay in range. Pad each (window, half) group to
    a multiple of 128 edges with zero-weight edges; tile counts are maxed
    across cores so all 8 cores run the identical (SPMD) program.
  - Device phase 1: XW = X @ W computed redundantly per core (TensorE bf16,
    fp32 PSUM), stored bf16 into a DRAM scratch with rows padded to 128 cols
    (256B - dma_gather's granularity) in a partition-major permuted order so
    the stores are a few large contiguous DMAs.
  - Device phase 2: dma_gather (SWDGE descriptor-per-edge) fetches the XW
    rows for each 128-edge tile; VectorE builds S[e, r] = val[e] *
    (row_local[e] == r) via an iota compare; TensorE accumulates S^T @ rows
    into the window's PSUM [128, 64]. Windows are written out dense - no
    scatter races anywhere.
  - Host: concatenate the 8 output shards.
"""

from contextlib import ExitStack

import ml_dtypes
import numpy as np

import concourse.bacc as bacc
import concourse.bass as bass
import concourse.mybir as mybir
import concourse.tile as tile
from concourse._compat import get_trn_type
from concourse.bass_utils import run_bass_kernel_spmd

N_NODES = 50000
N_EDGES = 800000
F_IN = 256
F_OUT = 64
N_CORES = 8
SHARD = N_NODES // N_CORES  # 6250 destination rows per core
WIN = 128  # destination rows per PSUM accumulation window
BF16 = ml_dtypes.bfloat16

# knobs
SLAB = 4096  # phase-1 node columns per XT slab DMA
GRP = 32  # phase-1 node tiles per staged XW store DMA
CH = 48  # phase-2 edge tiles (of 128 edges) per dma_gather call
GB = 8  # phase-2 edge tiles per batched one-hot / rhs-scale (divides CH)
# prepare_only+trigger would let desc-gen overlap phase 1, but Tile's
# consumer sync for gen_mode=1 preps fires at descriptor-write (not DMA
# landing) time, which corrupts results - keep off.
PREP = False
SIM_MEMSET = False  # zero staging tiles (only needed to appease CoreSim)

# test.py pokes these for profiling
TRACE = False
LAST_RESULTS = None


def _install_ntff_hook():
    """The agent image's antenv lacks axon_hooks, so bass_utils' trace=True
    path can't find the NTFF hook. Recreate the module and register the
    ctypes-based hook exactly as trn_agent_boot would."""
    import sys
    import types

    try:
        import antenv.axon_hooks  # noqa: F401

        return True
    except ImportError:
        pass
    try:
        import antenv
        from trn_agent_boot.trn_boot import _ntff_profile_via_ctypes

        mod = types.ModuleType("antenv.axon_hooks")
        mod._hook = None

        def set_axon_ntff_profile_hook(h):
            mod._hook = h

        def get_axon_ntff_profile_hook():
            return mod._hook

        mod.set_axon_ntff_profile_hook = set_axon_ntff_profile_hook
        mod.get_axon_ntff_profile_hook = get_axon_ntff_profile_hook
        sys.modules["antenv.axon_hooks"] = mod
        antenv.axon_hooks = mod
        hook = _ntff_profile_via_ctypes("/opt/axon/libaxon_pjrt.so")
        if hook is not None:
            set_axon_ntff_profile_hook(hook)
        return hook is not None
    except Exception as e:  # profiling is best-effort
        print(f"ntff hook install failed: {e}")
        return False


def _wrap16(stream_i16, n_tiles):
    """Wrapped+replicated dma_gather index layout: stream position i lives at
    partition i%16 (replicated to all 8 16-partition groups), slot i//16."""
    n = n_tiles * 128
    w = np.zeros((128, n // 16), dtype=np.int16)
    s = np.zeros(n, dtype=np.int16)
    s[: len(stream_i16)] = stream_i16
    blk = s.reshape(n // 16, 16).T  # [16, n//16]
    for g in range(8):
        w[g * 16 : (g + 1) * 16, :] = blk
    return w


def _prep(X, W, edge_row, edge_col, edge_vals):
    """Host-side sharding/marshalling.

    Returns (in_maps, T_lo, T_hi): per-window tile counts for the lo/hi
    source halves (identical across cores -> shared SPMD program).
    """
    XT = np.ascontiguousarray(X.T).astype(BF16)  # [F_IN, N_NODES]
    Wb = np.ascontiguousarray(W).astype(BF16)  # [F_IN, F_OUT]
    iota = np.tile(np.arange(WIN, dtype=np.float32), (128, GB))  # [128, GB*WIN]

    NT = (N_NODES + 127) // 128  # 391 node tiles; xw virtual rows = 128*NT
    HALF = 64 * NT  # virtual row where the hi table starts (25024)

    n_win = (SHARD + WIN - 1) // WIN
    core = edge_row // SHARD
    percore = []
    cnt_lo = np.zeros((N_CORES, n_win), dtype=np.int64)
    cnt_hi = np.zeros((N_CORES, n_win), dtype=np.int64)
    for p in range(N_CORES):
        m = core == p
        r = edge_row[m].astype(np.int64) - p * SHARD
        c = edge_col[m].astype(np.int64)
        v = edge_vals[m].astype(np.float32)
        q = (c % 128) * NT + c // 128  # permuted xw virtual row
        hi = q >= HALF
        w = r // WIN
        # order: (window, half) groups; stable within
        order = np.lexsort((hi, w))
        r, q, v, hi, w = r[order], q[order], v[order], hi[order], w[order]
        percore.append((r, q, v, hi, w))
        for wi in range(n_win):
            mw = w == wi
            cnt_lo[p, wi] = (mw & ~hi).sum()
            cnt_hi[p, wi] = (mw & hi).sum()

    T_lo = np.maximum(1, -(-cnt_lo.max(axis=0) // 128))
    T_hi = np.maximum(1, -(-cnt_hi.max(axis=0) // 128))
    J_lo, J_hi = int(T_lo.sum()), int(T_hi.sum())
    J = J_lo + J_hi
    lo_starts = np.concatenate([[0], np.cumsum(T_lo)])
    hi_starts = np.concatenate([[0], np.cumsum(T_hi)])

    in_maps = []
    for p in range(N_CORES):
        r, q, v, hi, w = percore[p]
        lo_q = np.zeros(J_lo * 128, dtype=np.int64)
        hi_q = np.zeros(J_hi * 128, dtype=np.int64)
        # consumption-order meta: per window, T_lo[w] lo tiles then T_hi[w] hi tiles
        vals = np.zeros(J * 128, dtype=np.float32)
        rowloc = np.zeros(J * 128, dtype=np.float32)
        for wi in range(n_win):
            for is_hi, starts_h, qbuf, Th in (
                (False, lo_starts, lo_q, T_lo),
                (True, hi_starts, hi_q, T_hi),
            ):
                mw = (w == wi) & (hi == is_hi)
                n = int(mw.sum())
                s0 = int(starts_h[wi]) * 128
                qq = q[mw] - (HALF if is_hi else 0)
                qbuf[s0 : s0 + n] = qq
                # meta position: window base + (hi ? T_lo[wi] : 0) tiles
                mb = (
                    int(lo_starts[wi]) + int(hi_starts[wi]) + (int(T_lo[wi]) if is_hi else 0)
                ) * 128
                vals[mb : mb + n] = v[mw]
                rowloc[mb : mb + n] = (r[mw] % WIN).astype(np.float32)
        meta = np.concatenate(
            [rowloc.reshape(J, 128).T, vals.reshape(J, 128).T, iota], axis=1
        )
        in_maps.append(
            {
                "xt": XT,
                "w": Wb,
                "cols_lo": _wrap16(lo_q.astype(np.int16), J_lo),
                "cols_hi": _wrap16(hi_q.astype(np.int16), J_hi),
                "meta": np.ascontiguousarray(meta),
            }
        )
    return in_maps, T_lo, T_hi


def _build_nc(T_lo, T_hi, n_nodes=N_NODES, f_in=F_IN, f_out=F_OUT, shard=SHARD):
    f32 = mybir.dt.float32
    bf16 = mybir.dt.bfloat16
    i16 = mybir.dt.int16
    n_win = len(T_lo)
    J_lo, J_hi = int(T_lo.sum()), int(T_hi.sum())
    J = J_lo + J_hi
    lo_starts = np.concatenate([[0], np.cumsum(T_lo)])
    hi_starts = np.concatenate([[0], np.cumsum(T_hi)])

    NT = (n_nodes + 127) // 128  # node tiles
    HALF_ROWS = 64 * NT  # rows per half table

    nc = bacc.Bacc(
        get_trn_type() or "TRN2",
        target_bir_lowering=False,
        dynamic_dma_scratch_size=32768,
    )
    xt = nc.dram_tensor("xt", [f_in, n_nodes], bf16, kind="ExternalInput")
    w_in = nc.dram_tensor("w", [f_in, f_out], bf16, kind="ExternalInput")
    cols_lo = nc.dram_tensor("cols_lo", [128, J_lo * 8], i16, kind="ExternalInput")
    cols_hi = nc.dram_tensor("cols_hi", [128, J_hi * 8], i16, kind="ExternalInput")
    meta = nc.dram_tensor("meta", [128, 2 * J + GB * WIN], f32, kind="ExternalInput")
    out = nc.dram_tensor("out", [shard, f_out], f32, kind="ExternalOutput")
    # XW scratch: virtual row p*NT + t = XW[t*128 + p], cols padded 64->128
    # so each row is 256B (dma_gather granularity).
    xw = nc.dram_tensor("xw", [128 * NT, 128], bf16, kind="Internal")

    n_kc = f_in // 128  # contraction chunks (2)

    with tile.TileContext(nc) as tc, ExitStack() as ctx:
        const = ctx.enter_context(tc.tile_pool(name="const", bufs=1))
        xt_pool = ctx.enter_context(tc.tile_pool(name="xtp", bufs=2))
        psum1 = ctx.enter_context(tc.tile_pool(name="psum1", bufs=6, space="PSUM"))
        xw_sb = ctx.enter_context(tc.tile_pool(name="xw_sb", bufs=2))
        gath = ctx.enter_context(tc.tile_pool(name="gath", bufs=2))
        s_pool = ctx.enter_context(tc.tile_pool(name="s_pool", bufs=4))
        rhs_pool = ctx.enter_context(tc.tile_pool(name="rhs_pool", bufs=4))
        psum2 = ctx.enter_context(tc.tile_pool(name="psum2", bufs=2, space="PSUM"))
        out_sb = ctx.enter_context(tc.tile_pool(name="out_sb", bufs=4))

        # resident constants
        w_t = []
        for k in range(n_kc):
            wt = const.tile([128, f_out], bf16, tag=f"w{k}")
            nc.sync.dma_start(out=wt[:], in_=w_in[k * 128 : (k + 1) * 128, :])
            w_t.append(wt)
        meta_t = const.tile([128, 2 * J + GB * WIN], f32, tag="meta")
        nc.sync.dma_start(out=meta_t[:], in_=meta[:, :])
        clo_t = const.tile([128, J_lo * 8], i16, tag="clo")
        nc.sync.dma_start(out=clo_t[:], in_=cols_lo[:, :])
        chi_t = const.tile([128, J_hi * 8], i16, tag="chi")
        nc.sync.dma_start(out=chi_t[:], in_=cols_hi[:, :])

        # ---- phase 1: xw = (X @ W) in bf16, partition-major, 128-padded ----
        xw_pm = xw[:, :].rearrange("(p t) f -> p (t f)", p=128)  # [128, NT*128]
        stg = None
        g0 = 0
        s0 = 0
        xts = []
        for nt_i in range(NT):
            n0 = nt_i * 128
            m = min(128, n_nodes - n0)
            if nt_i % (SLAB // 128) == 0:
                s0 = n0
                sl = min(SLAB, n_nodes - s0)
                xts = []
                for k in range(n_kc):
                    xtk = xt_pool.tile([128, SLAB], bf16, tag=f"xt{k}")
                    nc.sync.dma_start(
                        out=xtk[:, :sl],
                        in_=xt[k * 128 : (k + 1) * 128, s0 : s0 + sl],
                    )
                    xts.append(xtk)
            if nt_i % GRP == 0:
                g0 = nt_i
                stg = xw_sb.tile([128, GRP * 128], bf16, tag="stg")
                if SIM_MEMSET:  # garbage bytes are never consumed on HW
                    nc.gpsimd.memset(stg[:], 0)
            ps = psum1.tile([128, f_out], f32, tag="ps1")
            for k in range(n_kc):
                nc.tensor.matmul(
                    out=ps[:m, :],
                    lhsT=xts[k][:, n0 - s0 : n0 - s0 + m],
                    rhs=w_t[k][:],
                    start=(k == 0),
                    stop=(k == n_kc - 1),
                )
            loc = nt_i - g0
            nc.vector.tensor_copy(
                out=stg[:m, loc * 128 : loc * 128 + f_out], in_=ps[:m, :]
            )
            if nt_i == NT - 1 or (nt_i + 1) % GRP == 0:
                gn = nt_i + 1 - g0
                nc.sync.dma_start(
                    out=xw_pm[:, g0 * 128 : (g0 + gn) * 128],
                    in_=stg[:, : gn * 128],
                )

        # ---- phase 2: dma_gather + batched one-hot matmul segment-sum ----
        # meta column of a (window, half, k) tile in consumption order
        def meta_col(w, is_hi, k):
            return (
                int(lo_starts[w]) + int(hi_starts[w]) + (int(T_lo[w]) if is_hi else 0) + k
            )

        # stream-position -> meta column (needed for batched builds)
        m_of = [np.zeros(J_lo, dtype=np.int64), np.zeros(J_hi, dtype=np.int64)]
        for w in range(n_win):
            for is_hi, starts_h, Th in ((0, lo_starts, T_lo), (1, hi_starts, T_hi)):
                for k in range(int(Th[w])):
                    m_of[is_hi][int(starts_h[w]) + k] = meta_col(w, bool(is_hi), k)

        chunks = {}  # (is_hi, chunk_idx) -> gather tile
        batches = {}  # (is_hi, batch_idx) -> (S_b, rhs_b)

        def ensure_chunk(is_hi, tile_idx):
            ci = tile_idx // CH
            key = (is_hi, ci)
            if key in chunks:
                return chunks[key]
            J_h = J_hi if is_hi else J_lo
            cols_t = chi_t if is_hi else clo_t
            base = HALF_ROWS if is_hi else 0
            cn = min(CH, J_h - ci * CH)
            g = gath.tile([128, CH, 128], bf16, tag=f"g{int(is_hi)}")
            kw = {}
            sem = None
            if PREP:
                sem = nc.alloc_semaphore(f"gsem_{int(is_hi)}_{ci}")
                kw = dict(prepare_only=True, sem=sem)
            nc.gpsimd.dma_gather(
                out_ap=g[:, :cn, :],
                in_ap=xw[base : base + HALF_ROWS, :],
                idxs_ap=cols_t[:, ci * CH * 8 : (ci * CH + cn) * 8],
                num_idxs=cn * 128,
                num_idxs_reg=cn * 128,
                elem_size=128,
                single_packet=False,
                **kw,
            )
            if PREP:
                nc.gpsimd.trigger_dma(count=None)
            chunks[key] = (g, sem)
            return chunks[key]

        def ensure_batch(is_hi, tile_idx):
            bi = tile_idx // GB
            key = (is_hi, bi)
            if key in batches:
                return batches[key]
            J_h = J_hi if is_hi else J_lo
            b0 = bi * GB
            bn = min(GB, J_h - b0)
            g, gsem = ensure_chunk(is_hi, b0)
            gs = b0 - (b0 // CH) * CH  # batch offset within its chunk
            # meta columns of this batch are contiguous within a (window, half)
            # group but the batch may span groups; split into contiguous runs.
            cols = m_of[int(is_hi)][b0 : b0 + bn]
            S_b = s_pool.tile([128, GB, WIN], bf16, tag=f"S{int(is_hi)}")
            rhs_b = rhs_pool.tile([128, GB, f_out], bf16, tag=f"r{int(is_hi)}")
            runs = []
            r0 = 0
            for i in range(1, bn + 1):
                if i == bn or cols[i] != cols[i - 1] + 1:
                    runs.append((r0, i))
                    r0 = i
            for a, b in runs:
                n = b - a
                mc = int(cols[a])
                nc.vector.tensor_tensor(
                    out=S_b[:, a:b, :],
                    in0=meta_t[:, 2 * J : 2 * J + n * WIN].rearrange(
                        "p (b r) -> p b r", r=WIN
                    ),
                    in1=meta_t[:, mc : mc + n].to_broadcast([128, n, WIN]),
                    op=mybir.AluOpType.is_equal,
                )
                if gsem is not None:
                    # Tile's consumer sync for gen_mode=1 preps fires at
                    # descriptor-write time; gate the g read on the actual
                    # DMA-completion sem, atomically with the read.
                    with tc.tile_critical():
                        nc.vector.wait_ge(gsem, 16)
                        nc.vector.tensor_tensor(
                            out=rhs_b[:, a:b, :],
                            in0=g[:, gs + a : gs + b, 0:f_out],
                            in1=meta_t[:, J + mc : J + mc + n].to_broadcast(
                                [128, n, f_out]
                            ),
                            op=mybir.AluOpType.mult,
                        )
                else:
                    nc.vector.tensor_tensor(
                        out=rhs_b[:, a:b, :],
                        in0=g[:, gs + a : gs + b, 0:f_out],
                        in1=meta_t[:, J + mc : J + mc + n].to_broadcast(
                            [128, n, f_out]
                        ),
                        op=mybir.AluOpType.mult,
                    )
            batches[key] = (S_b, rhs_b)
            return batches[key]

        for w in range(n_win):
            cur_ps = psum2.tile([128, f_out], f32, tag="ps2")
            n_t = int(T_lo[w] + T_hi[w])
            ti = 0  # tile index within this window's consumption order
            for is_hi, starts_h, Th in ((False, lo_starts, T_lo), (True, hi_starts, T_hi)):
                for k in range(int(Th[w])):
                    t_s = int(starts_h[w]) + k  # stream position
                    S_b, rhs_b = ensure_batch(is_hi, t_s)
                    sl = t_s % GB
                    nc.tensor.matmul(
                        out=cur_ps[:],
                        lhsT=S_b[:, sl : sl + 1, :],
                        rhs=rhs_b[:, sl : sl + 1, :],
                        start=(ti == 0),
                        stop=(ti == n_t - 1),
                    )
                    ti += 1
            rows = min(WIN, shard - w * WIN)
            ot = out_sb.tile([128, f_out], f32, tag="ot")
            nc.vector.tensor_copy(out=ot[:rows, :], in_=cur_ps[:rows, :])
            nc.sync.dma_start(out=out[w * WIN : w * WIN + rows, :], in_=ot[:rows, :])
    nc.compile()
    return nc


def kernel(X, W, edge_row, edge_col, edge_vals):
    global LAST_RESULTS
    X = np.asarray(X, dtype=np.float32)
    W = np.asarray(W, dtype=np.float32)
    edge_row = np.asarray(edge_row, dtype=np.int32)
    edge_col = np.asarray(edge_col, dtype=np.int32)
    edge_vals = np.asarray(edge_vals, dtype=np.float32)

    in_maps, T_lo, T_hi = _prep(X, W, edge_row, edge_col, edge_vals)
    nc = _build_nc(T_lo, T_hi)
    trace = TRACE and _install_ntff_hook()
    res = run_bass_kernel_spmd(
        nc, in_maps, core_ids=list(range(N_CORES)), trace=trace
    )
    LAST_RESULTS = res
    out = np.concatenate([res.results[p]["out"] for p in range(N_CORES)], axis=0)
    return out.astype(np.float32)



# revision 6
# speedup vs baseline: 2.6487x; 2.6487x over previous
"""GCN layer on 8 TRN2 NeuronCores (Bass/Tile).

out = segment_sum(edge_vals[:,None] * (X @ W)[edge_col], edge_row, N)

Strategy (1D destination-node sharding):
  - Host: cast/transpose X -> XT bf16 (replicated to all 8 cores). Partition
    edges by destination shard (6250 rows/core), group by destination window
    (128 rows), and within each window split by source-half (lo/hi) so the
    int16 dma_gather indices stay in range. Pad each (window, half) group to
    a multiple of 128 edges with zero-weight edges; tile counts are maxed
    across cores so all 8 cores run the identical (SPMD) program.
  - Device phase 1: XW = X @ W computed redundantly per core (TensorE bf16,
    fp32 PSUM), stored bf16 into a DRAM scratch with rows padded to 128 cols
    (256B - dma_gather's granularity) in a partition-major permuted order so
    the stores are a few large contiguous DMAs.
  - Device phase 2: dma_gather (SWDGE descriptor-per-edge) fetches the XW
    rows for each 128-edge tile; VectorE builds S[e, r] = val[e] *
    (row_local[e] == r) via an iota compare; TensorE accumulates S^T @ rows
    into the window's PSUM [128, 64]. Windows are written out dense - no
    scatter races anywhere.
  - Host: concatenate the 8 output shards.
"""

from contextlib import ExitStack

import ml_dtypes
import numpy as np

import concourse.bacc as bacc
import concourse.bass as bass
import concourse.mybir as mybir
import concourse.tile as tile
from concourse._compat import get_trn_type
from concourse.bass_utils import run_bass_kernel_spmd

N_NODES = 50000
N_EDGES = 800000
F_IN = 256
F_OUT = 64
N_CORES = 8
SHARD = N_NODES // N_CORES  # 6250 destination rows per core
WIN = 128  # destination rows per PSUM accumulation window
BF16 = ml_dtypes.bfloat16

# knobs
SLAB = 4096  # phase-1 node columns per XT slab DMA
GRP = 32  # phase-1 node tiles per staged XW store DMA
CH = 12  # phase-2 edge tiles (of 128 edges) per dma_gather call
NQ = 4  # SWDGE queues used round-robin by gather chunks
GATH_BUFS = 8  # gather chunks in flight per (lo/hi) stream
GB = 6  # phase-2 edge tiles per batched one-hot / rhs-scale (divides CH)
# prepare_only+trigger would let desc-gen overlap phase 1, but Tile's
# consumer sync for gen_mode=1 preps fires at descriptor-write (not DMA
# landing) time, which corrupts results - keep off.
PREP = False
SIM_MEMSET = False  # zero staging tiles (only needed to appease CoreSim)

# test.py pokes these for profiling
TRACE = False
LAST_RESULTS = None


def _install_ntff_hook():
    """The agent image's antenv lacks axon_hooks, so bass_utils' trace=True
    path can't find the NTFF hook. Recreate the module and register the
    ctypes-based hook exactly as trn_agent_boot would."""
    import sys
    import types

    try:
        import antenv.axon_hooks  # noqa: F401

        return True
    except ImportError:
        pass
    try:
        import antenv
        from trn_agent_boot.trn_boot import _ntff_profile_via_ctypes

        mod = types.ModuleType("antenv.axon_hooks")
        mod._hook = None

        def set_axon_ntff_profile_hook(h):
            mod._hook = h

        def get_axon_ntff_profile_hook():
            return mod._hook

        mod.set_axon_ntff_profile_hook = set_axon_ntff_profile_hook
        mod.get_axon_ntff_profile_hook = get_axon_ntff_profile_hook
        sys.modules["antenv.axon_hooks"] = mod
        antenv.axon_hooks = mod
        hook = _ntff_profile_via_ctypes("/opt/axon/libaxon_pjrt.so")
        if hook is not None:
            set_axon_ntff_profile_hook(hook)
        return hook is not None
    except Exception as e:  # profiling is best-effort
        print(f"ntff hook install failed: {e}")
        return False


def _wrap16(stream_i16, n_tiles):
    """Wrapped+replicated dma_gather index layout: stream position i lives at
    partition i%16 (replicated to all 8 16-partition groups), slot i//16."""
    n = n_tiles * 128
    w = np.zeros((128, n // 16), dtype=np.int16)
    s = np.zeros(n, dtype=np.int16)
    s[: len(stream_i16)] = stream_i16
    blk = s.reshape(n // 16, 16).T  # [16, n//16]
    for g in range(8):
        w[g * 16 : (g + 1) * 16, :] = blk
    return w


def _prep(X, W, edge_row, edge_col, edge_vals):
    """Host-side sharding/marshalling.

    Returns (in_maps, T_lo, T_hi): per-window tile counts for the lo/hi
    source halves (identical across cores -> shared SPMD program).
    """
    XT = np.ascontiguousarray(X.T).astype(BF16)  # [F_IN, N_NODES]
    Wb = np.ascontiguousarray(W).astype(BF16)  # [F_IN, F_OUT]
    iota = np.tile(np.arange(WIN, dtype=np.float32), (128, GB))  # [128, GB*WIN]

    NT = (N_NODES + 127) // 128  # 391 node tiles; xw virtual rows = 128*NT
    HALF = 64 * NT  # virtual row where the hi table starts (25024)

    n_win = (SHARD + WIN - 1) // WIN
    core = edge_row // SHARD
    percore = []
    cnt_lo = np.zeros((N_CORES, n_win), dtype=np.int64)
    cnt_hi = np.zeros((N_CORES, n_win), dtype=np.int64)
    for p in range(N_CORES):
        m = core == p
        r = edge_row[m].astype(np.int64) - p * SHARD
        c = edge_col[m].astype(np.int64)
        v = edge_vals[m].astype(np.float32)
        q = (c % 128) * NT + c // 128  # permuted xw virtual row
        hi = q >= HALF
        w = r // WIN
        # order: (window, half) groups; stable within
        order = np.lexsort((hi, w))
        r, q, v, hi, w = r[order], q[order], v[order], hi[order], w[order]
        percore.append((r, q, v, hi, w))
        for wi in range(n_win):
            mw = w == wi
            cnt_lo[p, wi] = (mw & ~hi).sum()
            cnt_hi[p, wi] = (mw & hi).sum()

    T_lo = np.maximum(1, -(-cnt_lo.max(axis=0) // 128))
    T_hi = np.maximum(1, -(-cnt_hi.max(axis=0) // 128))
    J_lo, J_hi = int(T_lo.sum()), int(T_hi.sum())
    J = J_lo + J_hi
    lo_starts = np.concatenate([[0], np.cumsum(T_lo)])
    hi_starts = np.concatenate([[0], np.cumsum(T_hi)])

    in_maps = []
    for p in range(N_CORES):
        r, q, v, hi, w = percore[p]
        lo_q = np.zeros(J_lo * 128, dtype=np.int64)
        hi_q = np.zeros(J_hi * 128, dtype=np.int64)
        # consumption-order meta: per window, T_lo[w] lo tiles then T_hi[w] hi tiles
        vals = np.zeros(J * 128, dtype=np.float32)
        rowloc = np.zeros(J * 128, dtype=np.float32)
        for wi in range(n_win):
            for is_hi, starts_h, qbuf, Th in (
                (False, lo_starts, lo_q, T_lo),
                (True, hi_starts, hi_q, T_hi),
            ):
                mw = (w == wi) & (hi == is_hi)
                n = int(mw.sum())
                s0 = int(starts_h[wi]) * 128
                qq = q[mw] - (HALF if is_hi else 0)
                qbuf[s0 : s0 + n] = qq
                # meta position: window base + (hi ? T_lo[wi] : 0) tiles
                mb = (
                    int(lo_starts[wi]) + int(hi_starts[wi]) + (int(T_lo[wi]) if is_hi else 0)
                ) * 128
                vals[mb : mb + n] = v[mw]
                rowloc[mb : mb + n] = (r[mw] % WIN).astype(np.float32)
        meta = np.concatenate(
            [rowloc.reshape(J, 128).T, vals.reshape(J, 128).T, iota], axis=1
        )
        in_maps.append(
            {
                "xt": XT,
                "w": Wb,
                "cols_lo": _wrap16(lo_q.astype(np.int16), J_lo),
                "cols_hi": _wrap16(hi_q.astype(np.int16), J_hi),
                "meta": np.ascontiguousarray(meta),
            }
        )
    return in_maps, T_lo, T_hi


def _build_nc(T_lo, T_hi, n_nodes=N_NODES, f_in=F_IN, f_out=F_OUT, shard=SHARD):
    f32 = mybir.dt.float32
    bf16 = mybir.dt.bfloat16
    i16 = mybir.dt.int16
    n_win = len(T_lo)
    J_lo, J_hi = int(T_lo.sum()), int(T_hi.sum())
    J = J_lo + J_hi
    lo_starts = np.concatenate([[0], np.cumsum(T_lo)])
    hi_starts = np.concatenate([[0], np.cumsum(T_hi)])

    NT = (n_nodes + 127) // 128  # node tiles
    HALF_ROWS = 64 * NT  # rows per half table

    nc = bacc.Bacc(
        get_trn_type() or "TRN2",
        target_bir_lowering=False,
        dynamic_dma_scratch_size=32768,
        num_swdge_queues=NQ,
    )
    xt = nc.dram_tensor("xt", [f_in, n_nodes], bf16, kind="ExternalInput")
    w_in = nc.dram_tensor("w", [f_in, f_out], bf16, kind="ExternalInput")
    cols_lo = nc.dram_tensor("cols_lo", [128, J_lo * 8], i16, kind="ExternalInput")
    cols_hi = nc.dram_tensor("cols_hi", [128, J_hi * 8], i16, kind="ExternalInput")
    meta = nc.dram_tensor("meta", [128, 2 * J + GB * WIN], f32, kind="ExternalInput")
    out = nc.dram_tensor("out", [shard, f_out], f32, kind="ExternalOutput")
    # XW scratch: virtual row p*NT + t = XW[t*128 + p], cols padded 64->128
    # so each row is 256B (dma_gather granularity).
    xw = nc.dram_tensor("xw", [128 * NT, 128], bf16, kind="Internal")

    n_kc = f_in // 128  # contraction chunks (2)

    with tile.TileContext(nc) as tc, ExitStack() as ctx:
        const = ctx.enter_context(tc.tile_pool(name="const", bufs=1))
        xt_pool = ctx.enter_context(tc.tile_pool(name="xtp", bufs=2))
        psum1 = ctx.enter_context(tc.tile_pool(name="psum1", bufs=6, space="PSUM"))
        xw_sb = ctx.enter_context(tc.tile_pool(name="xw_sb", bufs=2))
        gath = ctx.enter_context(tc.tile_pool(name="gath", bufs=GATH_BUFS))
        s_pool = ctx.enter_context(tc.tile_pool(name="s_pool", bufs=4))
        rhs_pool = ctx.enter_context(tc.tile_pool(name="rhs_pool", bufs=4))
        psum2 = ctx.enter_context(tc.tile_pool(name="psum2", bufs=2, space="PSUM"))
        out_sb = ctx.enter_context(tc.tile_pool(name="out_sb", bufs=4))

        # resident constants
        w_t = []
        for k in range(n_kc):
            wt = const.tile([128, f_out], bf16, tag=f"w{k}")
            nc.sync.dma_start(out=wt[:], in_=w_in[k * 128 : (k + 1) * 128, :])
            w_t.append(wt)
        meta_t = const.tile([128, 2 * J + GB * WIN], f32, tag="meta")
        nc.sync.dma_start(out=meta_t[:], in_=meta[:, :])
        clo_t = const.tile([128, J_lo * 8], i16, tag="clo")
        nc.sync.dma_start(out=clo_t[:], in_=cols_lo[:, :])
        chi_t = const.tile([128, J_hi * 8], i16, tag="chi")
        nc.sync.dma_start(out=chi_t[:], in_=cols_hi[:, :])

        # ---- phase 1: xw = (X @ W) in bf16, partition-major, 128-padded ----
        xw_pm = xw[:, :].rearrange("(p t) f -> p (t f)", p=128)  # [128, NT*128]
        stg = None
        g0 = 0
        s0 = 0
        xts = []
        for nt_i in range(NT):
            n0 = nt_i * 128
            m = min(128, n_nodes - n0)
            if nt_i % (SLAB // 128) == 0:
                s0 = n0
                sl = min(SLAB, n_nodes - s0)
                xts = []
                for k in range(n_kc):
                    xtk = xt_pool.tile([128, SLAB], bf16, tag=f"xt{k}")
                    nc.sync.dma_start(
                        out=xtk[:, :sl],
                        in_=xt[k * 128 : (k + 1) * 128, s0 : s0 + sl],
                    )
                    xts.append(xtk)
            if nt_i % GRP == 0:
                g0 = nt_i
                stg = xw_sb.tile([128, GRP * 128], bf16, tag="stg")
                if SIM_MEMSET:  # garbage bytes are never consumed on HW
                    nc.gpsimd.memset(stg[:], 0)
            ps = psum1.tile([128, f_out], f32, tag="ps1")
            for k in range(n_kc):
                nc.tensor.matmul(
                    out=ps[:m, :],
                    lhsT=xts[k][:, n0 - s0 : n0 - s0 + m],
                    rhs=w_t[k][:],
                    start=(k == 0),
                    stop=(k == n_kc - 1),
                )
            loc = nt_i - g0
            nc.vector.tensor_copy(
                out=stg[:m, loc * 128 : loc * 128 + f_out], in_=ps[:m, :]
            )
            if nt_i == NT - 1 or (nt_i + 1) % GRP == 0:
                gn = nt_i + 1 - g0
                nc.sync.dma_start(
                    out=xw_pm[:, g0 * 128 : (g0 + gn) * 128],
                    in_=stg[:, : gn * 128],
                )

        # ---- phase 2: dma_gather + batched one-hot matmul segment-sum ----
        # meta column of a (window, half, k) tile in consumption order
        def meta_col(w, is_hi, k):
            return (
                int(lo_starts[w]) + int(hi_starts[w]) + (int(T_lo[w]) if is_hi else 0) + k
            )

        # stream-position -> meta column (needed for batched builds)
        m_of = [np.zeros(J_lo, dtype=np.int64), np.zeros(J_hi, dtype=np.int64)]
        for w in range(n_win):
            for is_hi, starts_h, Th in ((0, lo_starts, T_lo), (1, hi_starts, T_hi)):
                for k in range(int(Th[w])):
                    m_of[is_hi][int(starts_h[w]) + k] = meta_col(w, bool(is_hi), k)

        chunks = {}  # (is_hi, chunk_idx) -> gather tile
        batches = {}  # (is_hi, batch_idx) -> (S_b, rhs_b)
        issue_ctr = [0]  # round-robins gather chunks across SWDGE queues

        def ensure_chunk(is_hi, tile_idx):
            ci = tile_idx // CH
            key = (is_hi, ci)
            if key in chunks:
                return chunks[key]
            J_h = J_hi if is_hi else J_lo
            cols_t = chi_t if is_hi else clo_t
            base = HALF_ROWS if is_hi else 0
            cn = min(CH, J_h - ci * CH)
            g = gath.tile([128, CH, 128], bf16, tag=f"g{int(is_hi)}")
            kw = {}
            sem = None
            if PREP:
                sem = nc.alloc_semaphore(f"gsem_{int(is_hi)}_{ci}")
                kw = dict(prepare_only=True, sem=sem)
            nc.gpsimd.dma_gather(
                out_ap=g[:, :cn, :],
                in_ap=xw[base : base + HALF_ROWS, :],
                idxs_ap=cols_t[:, ci * CH * 8 : (ci * CH + cn) * 8],
                num_idxs=cn * 128,
                num_idxs_reg=cn * 128,
                elem_size=128,
                single_packet=False,
                queue_num=issue_ctr[0] % NQ,
                **kw,
            )
            issue_ctr[0] += 1
            if PREP:
                nc.gpsimd.trigger_dma(count=None)
            chunks[key] = (g, sem)
            return chunks[key]

        def ensure_batch(is_hi, tile_idx):
            bi = tile_idx // GB
            key = (is_hi, bi)
            if key in batches:
                return batches[key]
            J_h = J_hi if is_hi else J_lo
            b0 = bi * GB
            bn = min(GB, J_h - b0)
            g, gsem = ensure_chunk(is_hi, b0)
            gs = b0 - (b0 // CH) * CH  # batch offset within its chunk
            # meta columns of this batch are contiguous within a (window, half)
            # group but the batch may span groups; split into contiguous runs.
            cols = m_of[int(is_hi)][b0 : b0 + bn]
            S_b = s_pool.tile([128, GB, WIN], bf16, tag=f"S{int(is_hi)}")
            rhs_b = rhs_pool.tile([128, GB, f_out], bf16, tag=f"r{int(is_hi)}")
            runs = []
            r0 = 0
            for i in range(1, bn + 1):
                if i == bn or cols[i] != cols[i - 1] + 1:
                    runs.append((r0, i))
                    r0 = i
            for a, b in runs:
                n = b - a
                mc = int(cols[a])
                nc.vector.tensor_tensor(
                    out=S_b[:, a:b, :],
                    in0=meta_t[:, 2 * J : 2 * J + n * WIN].rearrange(
                        "p (b r) -> p b r", r=WIN
                    ),
                    in1=meta_t[:, mc : mc + n].to_broadcast([128, n, WIN]),
                    op=mybir.AluOpType.is_equal,
                )
                if gsem is not None:
                    # Tile's consumer sync for gen_mode=1 preps fires at
                    # descriptor-write time; gate the g read on the actual
                    # DMA-completion sem, atomically with the read.
                    with tc.tile_critical():
                        nc.vector.wait_ge(gsem, 16)
                        nc.vector.tensor_tensor(
                            out=rhs_b[:, a:b, :],
                            in0=g[:, gs + a : gs + b, 0:f_out],
                            in1=meta_t[:, J + mc : J + mc + n].to_broadcast(
                                [128, n, f_out]
                            ),
                            op=mybir.AluOpType.mult,
                        )
                else:
                    nc.vector.tensor_tensor(
                        out=rhs_b[:, a:b, :],
                        in0=g[:, gs + a : gs + b, 0:f_out],
                        in1=meta_t[:, J + mc : J + mc + n].to_broadcast(
                            [128, n, f_out]
                        ),
                        op=mybir.AluOpType.mult,
                    )
            batches[key] = (S_b, rhs_b)
            return batches[key]

        for w in range(n_win):
            cur_ps = psum2.tile([128, f_out], f32, tag="ps2")
            n_t = int(T_lo[w] + T_hi[w])
            ti = 0  # tile index within this window's consumption order
            for is_hi, starts_h, Th in ((False, lo_starts, T_lo), (True, hi_starts, T_hi)):
                for k in range(int(Th[w])):
                    t_s = int(starts_h[w]) + k  # stream position
                    S_b, rhs_b = ensure_batch(is_hi, t_s)
                    sl = t_s % GB
                    nc.tensor.matmul(
                        out=cur_ps[:],
                        lhsT=S_b[:, sl : sl + 1, :],
                        rhs=rhs_b[:, sl : sl + 1, :],
                        start=(ti == 0),
                        stop=(ti == n_t - 1),
                    )
                    ti += 1
            rows = min(WIN, shard - w * WIN)
            ot = out_sb.tile([128, f_out], f32, tag="ot")
            nc.vector.tensor_copy(out=ot[:rows, :], in_=cur_ps[:rows, :])
            nc.sync.dma_start(out=out[w * WIN : w * WIN + rows, :], in_=ot[:rows, :])
    nc.compile()
    return nc


def kernel(X, W, edge_row, edge_col, edge_vals):
    global LAST_RESULTS
    X = np.asarray(X, dtype=np.float32)
    W = np.asarray(W, dtype=np.float32)
    edge_row = np.asarray(edge_row, dtype=np.int32)
    edge_col = np.asarray(edge_col, dtype=np.int32)
    edge_vals = np.asarray(edge_vals, dtype=np.float32)

    in_maps, T_lo, T_hi = _prep(X, W, edge_row, edge_col, edge_vals)
    nc = _build_nc(T_lo, T_hi)
    trace = TRACE and _install_ntff_hook()
    res = run_bass_kernel_spmd(
        nc, in_maps, core_ids=list(range(N_CORES)), trace=trace
    )
    LAST_RESULTS = res
    out = np.concatenate([res.results[p]["out"] for p in range(N_CORES)], axis=0)
    return out.astype(np.float32)

